# revision 21
# baseline (speedup 1.0000x reference)
"""Trainium2 Bass kernel for nn_CombinedLoss (LCCNet CombinedLoss).

Strategy
--------
The only heavy part is the point-cloud term: for each sample b,
    err_n = || (RT_inv_b - I) @ p_n ||   over N=200000 homogeneous points,
    loss_pc = sum_b mean_n err_n.
Everything else (pose loss, 4x4 transform algebra) is O(B) scalar work done
on the host in float64.

Per sample the displacement is d = A3 @ q + a4 with q = (x,y,z). Using a
column-pivoted QR A3 = Q R (orthogonal Q preserves the norm):
    err^2 = (s1*(u1 + a*u2 + b*u3) + b1)^2
          + (s2*(u2 + g*u3) + b2)^2
          + b3^2                      (A3 is rank 2 -> R[2,2] = 0)
where u = permuted coords, s_i = R[i,i], (b1,b2,b3) = Q^T a4, and the ratios
a,b,g are bounded by 1 thanks to pivoting (fp16-safe).

Device mapping (8 cores, data-parallel over batch; per core 4 samples packed
as 128 partitions = 4 samples x 32 partition-rows, 6250 points per row):
  - The per-sample scalars live in a [128, 1] column each (constant within a
    sample's 32 partitions), so ONE instruction covers all 4 samples.
  - DMA (gpsimd SWDGE, fp32->fp16 cast in flight) streams the free dim in
    chunks; compute starts when chunk 0 lands.
  - DVE per chunk: 3 tensor_scalar multiplies (4x perf mode) + 3
    tensor_tensor adds (2x mode) for the combines, + 1 add for s12.
    Instruction order keeps >=1 op between same-engine RAW pairs, so no
    drain() is needed.
  - ACT per chunk: 2 Square activations (fused per-partition scale+bias) +
    1 Sqrt with per-partition bias b3^2 and free-dim accum_out.
  - SP: waits for all Sqrts, DMAs the [128, n_chunks] accumulator out.
  - Host: final sums in float64, pose loss, combine.
"""

import numpy as np

B = 32
N = 200000
NCORES = 8
SPC = B // NCORES          # samples per core
NPART = 128
ROWS = 32                  # partition-rows per sample
PPTS = N // ROWS           # points per partition-row = 6250
# free-dim compute chunks (sum = PPTS, all even); small first chunk so
# compute starts early, small last chunk so the post-DMA tail is short.
CHUNKS = [768, 1024, 1248, 1248, 1024, 744, 194]
# each chunk's DMA is split into pieces issued as separate dma_starts: a
# single dma_start only engages ~5 of 16 SDMA queues (~140 GB/s), so
# concurrent pieces raise both aggregate bandwidth and per-chunk landing
# speed. Emission costs ~640ns/instruction, so pieces stay moderate.
PIECES = [[384, 384], [512, 512], [624, 624], [624, 624], [512, 512],
          [744], [194]]
NCHUNK = len(CHUNKS)
OFFS = [0]
for _f in CHUNKS:
    OFFS.append(OFFS[-1] + _f)
# sqrt grouping: (first_chunk, last_chunk) half-open chunk ranges, one
# accumulator column each; arranged so only narrow sqrts land in the tail
SQRT_GROUPS = [(0, 2), (2, 4), (4, 5), (5, 7)]
NCONST = 8                 # per-sample scalar constants

_CACHED_NC = None


def _quat_to_rot(q):
    """Normalized quaternion [w,x,y,z] -> 3x3 rotation matrix (float64)."""
    q = q / np.linalg.norm(q)
    w, x, y, z = q
    return np.array([
        [1 - 2*y*y - 2*z*z, 2*x*y - 2*z*w,     2*x*z + 2*y*w],
        [2*x*y + 2*z*w,     1 - 2*x*x - 2*z*z, 2*y*z - 2*x*w],
        [2*x*z - 2*y*w,     2*y*z + 2*x*w,     1 - 2*x*x - 2*y*y],
    ])


def _pivoted_qr(A3):
    """Column-pivoted QR of a 3x3 matrix (float64). A3[:, piv] = Q @ R.

    Modified Gram-Schmidt with greedy max-residual-norm pivoting, which
    guarantees |R[i, j]| <= |R[i, i]| for j > i (bounded ratios)."""
    cols = {c: A3[:, c].astype(np.float64).copy() for c in range(3)}
    coeff = {c: np.zeros(3) for c in range(3)}   # coeff[c][i] = Q[:,i].A3[:,c]
    remaining = [0, 1, 2]
    piv = []
    Q = np.zeros((3, 3))
    for i in range(3):
        cbest = max(remaining, key=lambda c: float(np.dot(cols[c], cols[c])))
        remaining.remove(cbest)
        piv.append(cbest)
        v = cols[cbest]
        nrm = np.sqrt(np.dot(v, v))
        if nrm < 1e-300:
            # Degenerate column: pick any unit vector orthogonal to prior qs.
            for basis in np.eye(3):
                w = basis - Q[:, :i] @ (Q[:, :i].T @ basis)
                if np.dot(w, w) > 1e-12:
                    v = w
                    break
            nrm = np.sqrt(np.dot(v, v))
        q = v / nrm
        Q[:, i] = q
        for c in [cbest] + remaining:
            proj = float(np.dot(q, cols[c]))
            coeff[c][i] = proj
            cols[c] = cols[c] - proj * q
    R = np.stack([coeff[c] for c in piv], axis=1)
    return Q, R, piv


def _per_sample_host(tt, tr, te, re_):
    """Returns (piv, consts[8] float32) for one sample."""
    R_t = _quat_to_rot(tr.astype(np.float64))
    R_p = _quat_to_rot(re_.astype(np.float64))
    A3 = R_p.T @ R_t - np.eye(3)
    a4 = R_p.T @ (tt.astype(np.float64) - te.astype(np.float64))
    Q, R, piv = _pivoted_qr(A3)
    b4 = Q.T @ a4
    r11, r12, r13 = R[0, 0], R[0, 1], R[0, 2]
    r22, r23 = R[1, 1], R[1, 2]
    alpha = r12 / r11 if abs(r11) > 1e-30 else 0.0
    beta = r13 / r11 if abs(r11) > 1e-30 else 0.0
    gamma = r23 / r22 if abs(r22) > 1e-30 else 0.0
    consts = np.array([alpha, beta, gamma, r11, b4[0], r22, b4[1],
                       b4[2] ** 2])
    return piv, consts


def _build_nc():
    """Raw-Bass kernel (no TileContext): all synchronization is standalone
    wait_ge instructions on explicit semaphores.

    Engine programs:
      Pool (SWDGE): consts DMA, then one point-cloud DMA per free-dim chunk
          (fp32->fp16 cast in flight), each signalling its own semaphore.
          SWDGE emits descriptors in FIFO order -> chunks land in order.
      DVE: per chunk, 3 tensor_scalar multiplies + 3 tensor_tensor adds
          (combines; f1 signals), plus the s12 = t1 + t2 add (signals).
          s12 of chunk k is emitted after the combines of chunk k+1 so DVE
          never stalls on ACT, and every same-engine RAW pair has >=1
          instruction in between (no drain needed).
      ACT: per chunk, 2 Square activations (fused per-partition scale+bias;
          t2 signals) and 1 Sqrt with bias b3^2 and accum_out -> acc column
          (signals).
      SP: waits for all Sqrt results, DMAs acc out, waits for completion.
    """
    import concourse.bass as bass
    from concourse import mybir
    from contextlib import ExitStack

    f16, f32 = mybir.dt.float16, mybir.dt.float32
    Alu = mybir.AluOpType
    Act = mybir.ActivationFunctionType

    nc = bass.Bass("TRN2", target_bir_lowering=False, debug=False,
                   num_devices=NCORES)
    # piece-major flat layout: piece j of chunk k stores 3 contiguous
    # [128, Pf] blocks (one per coord), so every dma_start reads one
    # sequential HBM range.
    pc = nc.dram_tensor("pc", [NPART * 3 * PPTS], f32,
                        kind="ExternalInput").ap()
    # per-partition scalars [128, 8]: (alpha, beta, gamma, s1, b1, s2, b2,
    # b3sq), fp32; constant within each sample's 32 partitions.
    consts = nc.dram_tensor("consts", [NPART, NCONST], f32,
                            kind="ExternalInput").ap()
    ngroups = len(SQRT_GROUPS)
    acc_out0 = nc.dram_tensor("acc0", [NPART, ngroups - 1], f32,
                              kind="ExternalOutput").ap()
    acc_out1 = nc.dram_tensor("acc1", [NPART, 1], f32,
                              kind="ExternalOutput").ap()

    with ExitStack() as ctx:
        E = ctx.enter_context
        ct = E(nc.sbuf_tensor("ct", [NPART, NCONST], f32))
        acc = E(nc.sbuf_tensor("acc_sb", [NPART, ngroups], f32))
        scr = E(nc.sbuf_tensor("scr", [NPART, 4], f16))

        def tiles(nm, mult=1):
            return [E(nc.sbuf_tensor(f"{nm}{k}", [NPART, mult * CHUNKS[k]],
                                     f16)) for k in range(NCHUNK)]

        us = tiles("u", 3)
        a1s = tiles("a1_")
        a2s = tiles("a2_")
        a3s = tiles("a3_")
        c1s = tiles("c1_")
        c2s = tiles("c2_")
        f1s = tiles("f1_")
        t1s = tiles("t1_")
        t2s = tiles("t2_")
        # s12 results for all chunks live in ONE tile so sqrt can span
        # several chunks in a single activation
        s12all = E(nc.sbuf_tensor("s12all", [NPART, PPTS], f16))
        esall = E(nc.sbuf_tensor("esall", [NPART, PPTS], f16))

        sem_u = [E(nc.semaphore(f"sem_u{k}")) for k in range(NCHUNK)]
        sem_dve = E(nc.semaphore("sem_dve"))
        sem_act = E(nc.semaphore("sem_act"))
        sem_out = E(nc.semaphore("sem_out"))
        block = E(nc.Block())

        def cst(i):
            return ct[:, i:i + 1]

        def s12sl(k):
            return s12all[:, OFFS[k]:OFFS[k + 1]]

        # --- engine emit orders ---
        # DVE: groups in chunk order; s12_j two chunks behind (its wait on
        # ACT is then always satisfied), with the last three s12s pulled
        # forward/trailing so the tail sqrt groups unblock fast.
        dve_s12_after = {k: [] for k in range(NCHUNK)}
        for j in range(NCHUNK - 4):
            dve_s12_after[j + 2].append(j)
        dve_s12_after[NCHUNK - 2] += [NCHUNK - 4, NCHUNK - 3]
        dve_s12_after[NCHUNK - 1] += [NCHUNK - 2, NCHUNK - 1]
        # ACT: squares per chunk; sqrt group g after squares of chunk
        # grp_after[g] (chosen so its s12s are already emitted on DVE)
        grp_after = {}
        for gi, (a, bb) in enumerate(SQRT_GROUPS):
            if bb >= NCHUNK:
                grp_after[gi] = NCHUNK      # trailing, after every square
            elif bb + 1 >= NCHUNK - 1:
                # near the end: don't gate this sqrt on the last chunk's
                # squares (the last chunk lands latest)
                grp_after[gi] = bb
            else:
                grp_after[gi] = bb + 1      # one chunk of slack
        # groups whose last s12 only appears after the final G: trailing
        act_order = []
        for k in range(NCHUNK):
            act_order.append(("sq", k))
            for gi, (a, bb) in enumerate(SQRT_GROUPS):
                if grp_after[gi] == k and bb <= k:
                    act_order.append(("grp", gi))
        emitted = {x[1] for x in act_order if x[0] == "grp"}
        for gi in range(len(SQRT_GROUPS)):
            if gi not in emitted:
                act_order.append(("grp", gi))

        # --- semaphore tick bookkeeping (program order per engine) ---
        dve_c2, dve_f1, dve_s12 = {}, {}, {}
        act_t1, act_grp = {}, {}
        dve_n = act_n = 0
        for k in range(NCHUNK):
            dve_n += 1; dve_c2[k] = dve_n
            dve_n += 1; dve_f1[k] = dve_n
            for j in dve_s12_after[k]:
                dve_n += 1; dve_s12[j] = dve_n
        for op, idx in act_order:
            act_n += 1
            if op == "sq":
                act_t1[idx] = act_n
            else:
                act_grp[idx] = act_n

        @block.gpsimd
        def _(g):
            g.dma_start(ct[:], consts).then_inc(sem_u[0], 16)
            for k in range(NCHUNK):
                F = CHUNKS[k]
                poff = 0
                for Pf in PIECES[k]:
                    base = NPART * 3 * (OFFS[k] + poff)
                    for c in range(3):
                        g.dma_start(
                            us[k][:, c * F + poff:c * F + poff + Pf],
                            pc[base + c * NPART * Pf:
                               base + (c + 1) * NPART * Pf].rearrange(
                                "(p f) -> p f", p=NPART),
                        ).then_inc(sem_u[k], 16)
                    poff += Pf

        @block.vector
        def _(v):
            def combines(k):
                F = CHUNKS[k]
                u = us[k]
                u1, u2, u3 = (u[:, i * F:(i + 1) * F] for i in range(3))
                # chunk k's sem counts 16 per piece DMA x 3 coords
                # (+16 for the consts DMA on chunk 0)
                need = 48 * len(PIECES[k]) + (16 if k == 0 else 0)
                v.wait_ge(sem_u[k], need)
                # order keeps >=1 instruction between each same-engine RAW
                # pair: a2->c1 dist 2, a3->c2 dist 4, c1->f1 dist 2.
                v.tensor_scalar(a3s[k][:], u3, cst(2), None, Alu.mult)
                v.tensor_scalar(a2s[k][:], u3, cst(1), None, Alu.mult)
                v.tensor_scalar(a1s[k][:], u2, cst(0), None, Alu.mult)
                v.tensor_tensor(c1s[k][:], u1, a2s[k][:], Alu.add)
                v.tensor_tensor(c2s[k][:], u2, a3s[k][:],
                                Alu.add).then_inc(sem_dve, 1)
                v.tensor_tensor(f1s[k][:], c1s[k][:], a1s[k][:],
                                Alu.add).then_inc(sem_dve, 1)

            def s12(k):
                v.wait_ge(sem_act, act_t1[k])
                v.tensor_tensor(s12sl(k), t1s[k][:], t2s[k][:],
                                Alu.add).then_inc(sem_dve, 1)

            for k in range(NCHUNK):
                combines(k)
                for j in dve_s12_after[k]:
                    s12(j)

        @block.scalar
        def _(s):
            # dummy activations preload the Square/Sqrt tables while the
            # first DMA is still in flight
            s.activation(scr[:, 2:4], scr[:, 0:2], Act.Square)
            s.activation(scr[:, 0:2], scr[:, 0:2], Act.Sqrt)

            def squares(k):
                s.wait_ge(sem_dve, dve_c2[k])
                s.activation(t2s[k][:], c2s[k][:], Act.Square,
                             bias=cst(6), scale=cst(5))
                s.wait_ge(sem_dve, dve_f1[k])
                s.activation(t1s[k][:], f1s[k][:], Act.Square,
                             bias=cst(4), scale=cst(3)).then_inc(sem_act, 1)

            def sqrt_grp(gi):
                a, bb = SQRT_GROUPS[gi]
                dve_need = max(dve_s12[k] for k in range(a, bb))
                s.wait_ge(sem_dve, dve_need)
                s.activation(esall[:, OFFS[a]:OFFS[bb]],
                             s12all[:, OFFS[a]:OFFS[bb]], Act.Sqrt,
                             bias=cst(7),
                             accum_out=acc[:, gi:gi + 1]).then_inc(sem_act, 1)

            for op, idx in act_order:
                if op == "sq":
                    squares(idx)
                else:
                    sqrt_grp(idx)

        @block.sync
        def _(sp):
            # first half of the accumulator goes out as soon as its sqrt
            # groups are done; the rest right at the end
            half = ngroups - 1
            sp.wait_ge(sem_act, act_grp[half - 1])
            sp.dma_start(acc_out0, acc[:, 0:half]).then_inc(sem_out, 16)
            sp.wait_ge(sem_act, act_grp[ngroups - 1])
            sp.dma_start(acc_out1, acc[:, half:]).then_inc(sem_out, 16)
            sp.wait_ge(sem_out, 32)

    return nc


def _get_nc():
    global _CACHED_NC
    if _CACHED_NC is None:
        _CACHED_NC = _build_nc()
    return _CACHED_NC


def _kernel_impl(point_clouds, target_transl, target_rot, transl_err, rot_err,
                 trace=False):
    from concourse.bass_utils import run_bass_kernel_spmd

    pc = np.asarray(point_clouds)
    tt = np.asarray(target_transl, np.float64)
    tr = np.asarray(target_rot, np.float64)
    te = np.asarray(transl_err, np.float64)
    re_ = np.asarray(rot_err, np.float64)

    # ---- pose loss (host, float64, exact reference formulas) ----
    d = np.abs(te - tt)
    loss_transl = np.where(d < 1.0, 0.5 * d * d, d - 0.5).sum(axis=1).mean()

    rinv = tr * np.array([1.0, -1.0, -1.0, -1.0])
    q = re_
    w = q[:, 0]*rinv[:, 0] - q[:, 1]*rinv[:, 1] - q[:, 2]*rinv[:, 2] - q[:, 3]*rinv[:, 3]
    x = q[:, 0]*rinv[:, 1] + q[:, 1]*rinv[:, 0] + q[:, 2]*rinv[:, 3] - q[:, 3]*rinv[:, 2]
    y = q[:, 0]*rinv[:, 2] - q[:, 1]*rinv[:, 3] + q[:, 2]*rinv[:, 0] + q[:, 3]*rinv[:, 1]
    z = q[:, 0]*rinv[:, 3] + q[:, 1]*rinv[:, 2] - q[:, 2]*rinv[:, 1] + q[:, 3]*rinv[:, 0]
    angle = 2.0 * np.arctan2(np.sqrt(x*x + y*y + z*z), np.abs(w))
    loss_rot = (180.0 * angle / np.pi).mean()
    pose_loss = loss_transl + loss_rot

    # ---- per-sample transform constants (host) ----
    all_consts = np.zeros((B, NCONST), np.float32)
    all_piv = []
    for b in range(B):
        piv, consts = _per_sample_host(tt[b], tr[b], te[b], re_[b])
        all_consts[b] = consts
        all_piv.append(piv)

    # ---- build per-core inputs (permute coord rows per pivoting, pack
    #      4 samples x 32 partition-rows x 6250 points, chunk-major) ----
    pcp = np.stack([pc[b, all_piv[b], :] for b in range(B)])   # [B,3,N]
    pcp = pcp.reshape(NCORES, SPC, 3, ROWS, PPTS).transpose(0, 1, 3, 2, 4)
    pcp = pcp.reshape(NCORES, NPART, 3, PPTS)
    # piece-major flat: [concat over (chunk, piece) of [3, 128, Pf]]
    blocks = []
    for k in range(NCHUNK):
        poff = OFFS[k]
        for Pf in PIECES[k]:
            blocks.append(pcp[:, :, :, poff:poff + Pf]
                          .transpose(0, 2, 1, 3).reshape(NCORES, -1))
            poff += Pf
    pcf = np.concatenate(blocks, axis=1)
    in_maps = []
    for k in range(NCORES):
        cc = all_consts[k * SPC:(k + 1) * SPC]                 # [SPC, 8]
        in_maps.append({
            "pc": np.ascontiguousarray(pcf[k]),
            "consts": np.repeat(cc, ROWS, axis=0),             # [128, 8]
        })

    nc = _get_nc()
    res = run_bass_kernel_spmd(nc, in_maps, core_ids=list(range(NCORES)),
                               trace=trace)

    # ---- combine (host, float64) ----
    pcl_sum = 0.0
    for k in range(NCORES):
        pcl_sum += (res.results[k]["acc0"].astype(np.float64).sum()
                    + res.results[k]["acc1"].astype(np.float64).sum()) / N

    total = 0.5 * pose_loss + 0.5 * (pcl_sum / B)
    out = (np.float32(total), np.float32(loss_transl), np.float32(loss_rot),
           np.float32(pcl_sum / B))
    return out, res


def kernel(point_clouds, target_transl, target_rot, transl_err, rot_err):
    out, _ = _kernel_impl(point_clouds, target_transl, target_rot,
                          transl_err, rot_err)
    return out


# revision 26
# speedup vs baseline: 1.0213x; 1.0213x over previous
"""Trainium2 Bass kernel for nn_CombinedLoss (LCCNet CombinedLoss).

Strategy
--------
The only heavy part is the point-cloud term: for each sample b,
    err_n = || (RT_inv_b - I) @ p_n ||   over N=200000 homogeneous points,
    loss_pc = sum_b mean_n err_n.
Everything else (pose loss, 4x4 transform algebra) is O(B) scalar work done
on the host in float64.

Per sample the displacement is d = A3 @ q + a4 with q = (x,y,z). Using a
column-pivoted QR A3 = Q R (orthogonal Q preserves the norm):
    err^2 = (s1*(u1 + a*u2 + b*u3) + b1)^2
          + (s2*(u2 + g*u3) + b2)^2
          + b3^2                      (A3 is rank 2 -> R[2,2] = 0)
where u = permuted coords, s_i = R[i,i], (b1,b2,b3) = Q^T a4, and the ratios
a,b,g are bounded by 1 thanks to pivoting (fp16-safe).

Device mapping (8 cores, data-parallel over batch; per core 4 samples packed
as 128 partitions = 4 samples x 32 partition-rows, 6250 points per row):
  - The per-sample scalars live in a [128, 1] column each (constant within a
    sample's 32 partitions), so ONE instruction covers all 4 samples.
  - DMA (gpsimd SWDGE, fp32->fp16 cast in flight) streams the free dim in
    chunks; compute starts when chunk 0 lands.
  - DVE per chunk: 3 tensor_scalar multiplies (4x perf mode) + 3
    tensor_tensor adds (2x mode) for the combines, + 1 add for s12.
    Instruction order keeps >=1 op between same-engine RAW pairs, so no
    drain() is needed.
  - ACT per chunk: 2 Square activations (fused per-partition scale+bias) +
    1 Sqrt with per-partition bias b3^2 and free-dim accum_out.
  - SP: waits for all Sqrts, DMAs the [128, n_chunks] accumulator out.
  - Host: final sums in float64, pose loss, combine.
"""

import numpy as np

B = 32
N = 200000
NCORES = 8
SPC = B // NCORES          # samples per core
NPART = 128
ROWS = 32                  # partition-rows per sample
PPTS = N // ROWS           # points per partition-row = 6250
# free-dim compute chunks (sum = PPTS, all even); small first chunk so
# compute starts early, small last chunk so the post-DMA tail is short.
CHUNKS = [512, 896, 1152, 1248, 1152, 1096, 194]
# each chunk's DMA is split into f-range pieces issued as separate
# dma_starts (each covering all 3 coord rows = 384 descriptors): a single
# dma_start only engages ~5 of 16 SDMA queues (~140 GB/s), so two
# concurrent pieces double the per-chunk landing speed. Descriptor
# emission costs ~640ns per dma_start regardless of size, so pieces stay
# few and moderate.
PIECES = [[256, 256], [448, 448], [576, 576], [624, 624], [576, 576],
          [548, 548], [194]]
NCHUNK = len(CHUNKS)
OFFS = [0]
for _f in CHUNKS:
    OFFS.append(OFFS[-1] + _f)
# sqrt grouping: (first_chunk, last_chunk) half-open chunk ranges, one
# accumulator column each; arranged so only a narrow sqrt lands in the tail
SQRT_GROUPS = [(0, 2), (2, 4), (4, 6), (6, 7)]
NCONST = 8                 # per-sample scalar constants

_CACHED_NC = None


def _quat_to_rot(q):
    """Normalized quaternion [w,x,y,z] -> 3x3 rotation matrix (float64)."""
    q = q / np.linalg.norm(q)
    w, x, y, z = q
    return np.array([
        [1 - 2*y*y - 2*z*z, 2*x*y - 2*z*w,     2*x*z + 2*y*w],
        [2*x*y + 2*z*w,     1 - 2*x*x - 2*z*z, 2*y*z - 2*x*w],
        [2*x*z - 2*y*w,     2*y*z + 2*x*w,     1 - 2*x*x - 2*y*y],
    ])


def _pivoted_qr(A3):
    """Column-pivoted QR of a 3x3 matrix (float64). A3[:, piv] = Q @ R.

    Modified Gram-Schmidt with greedy max-residual-norm pivoting, which
    guarantees |R[i, j]| <= |R[i, i]| for j > i (bounded ratios)."""
    cols = {c: A3[:, c].astype(np.float64).copy() for c in range(3)}
    coeff = {c: np.zeros(3) for c in range(3)}   # coeff[c][i] = Q[:,i].A3[:,c]
    remaining = [0, 1, 2]
    piv = []
    Q = np.zeros((3, 3))
    for i in range(3):
        cbest = max(remaining, key=lambda c: float(np.dot(cols[c], cols[c])))
        remaining.remove(cbest)
        piv.append(cbest)
        v = cols[cbest]
        nrm = np.sqrt(np.dot(v, v))
        if nrm < 1e-300:
            # Degenerate column: pick any unit vector orthogonal to prior qs.
            for basis in np.eye(3):
                w = basis - Q[:, :i] @ (Q[:, :i].T @ basis)
                if np.dot(w, w) > 1e-12:
                    v = w
                    break
            nrm = np.sqrt(np.dot(v, v))
        q = v / nrm
        Q[:, i] = q
        for c in [cbest] + remaining:
            proj = float(np.dot(q, cols[c]))
            coeff[c][i] = proj
            cols[c] = cols[c] - proj * q
    R = np.stack([coeff[c] for c in piv], axis=1)
    return Q, R, piv


def _per_sample_host(tt, tr, te, re_):
    """Returns (piv, consts[8] float32) for one sample."""
    R_t = _quat_to_rot(tr.astype(np.float64))
    R_p = _quat_to_rot(re_.astype(np.float64))
    A3 = R_p.T @ R_t - np.eye(3)
    a4 = R_p.T @ (tt.astype(np.float64) - te.astype(np.float64))
    Q, R, piv = _pivoted_qr(A3)
    b4 = Q.T @ a4
    r11, r12, r13 = R[0, 0], R[0, 1], R[0, 2]
    r22, r23 = R[1, 1], R[1, 2]
    alpha = r12 / r11 if abs(r11) > 1e-30 else 0.0
    beta = r13 / r11 if abs(r11) > 1e-30 else 0.0
    gamma = r23 / r22 if abs(r22) > 1e-30 else 0.0
    consts = np.array([alpha, beta, gamma, r11, b4[0], r22, b4[1],
                       b4[2] ** 2])
    return piv, consts


def _build_nc():
    """Raw-Bass kernel (no TileContext): all synchronization is standalone
    wait_ge instructions on explicit semaphores.

    Engine programs:
      Pool (SWDGE): consts DMA, then one point-cloud DMA per free-dim chunk
          (fp32->fp16 cast in flight), each signalling its own semaphore.
          SWDGE emits descriptors in FIFO order -> chunks land in order.
      DVE: per chunk, 3 tensor_scalar multiplies + 3 tensor_tensor adds
          (combines; f1 signals), plus the s12 = t1 + t2 add (signals).
          s12 of chunk k is emitted after the combines of chunk k+1 so DVE
          never stalls on ACT, and every same-engine RAW pair has >=1
          instruction in between (no drain needed).
      ACT: per chunk, 2 Square activations (fused per-partition scale+bias;
          t2 signals) and 1 Sqrt with bias b3^2 and accum_out -> acc column
          (signals).
      SP: waits for all Sqrt results, DMAs acc out, waits for completion.
    """
    import concourse.bass as bass
    from concourse import mybir
    from contextlib import ExitStack

    f16, f32 = mybir.dt.float16, mybir.dt.float32
    Alu = mybir.AluOpType
    Act = mybir.ActivationFunctionType

    nc = bass.Bass("TRN2", target_bir_lowering=False, debug=False,
                   num_devices=NCORES)
    # piece-major flat layout: piece j of chunk k stores 3 contiguous
    # [128, Pf] blocks (one per coord), so every dma_start reads one
    # sequential HBM range.
    pc = nc.dram_tensor("pc", [NPART * 3 * PPTS], f32,
                        kind="ExternalInput").ap()
    # per-partition scalars [128, 8]: (alpha, beta, gamma, s1, b1, s2, b2,
    # b3sq), fp32; constant within each sample's 32 partitions.
    consts = nc.dram_tensor("consts", [NPART, NCONST], f32,
                            kind="ExternalInput").ap()
    ngroups = len(SQRT_GROUPS)
    acc_out0 = nc.dram_tensor("acc0", [NPART, ngroups - 1], f32,
                              kind="ExternalOutput").ap()
    acc_out1 = nc.dram_tensor("acc1", [NPART, 1], f32,
                              kind="ExternalOutput").ap()

    with ExitStack() as ctx:
        E = ctx.enter_context
        ct = E(nc.sbuf_tensor("ct", [NPART, NCONST], f32))
        acc = E(nc.sbuf_tensor("acc_sb", [NPART, ngroups], f32))
        scr = E(nc.sbuf_tensor("scr", [NPART, 4], f16))

        def tiles(nm, mult=1):
            return [E(nc.sbuf_tensor(f"{nm}{k}", [NPART, mult * CHUNKS[k]],
                                     f16)) for k in range(NCHUNK)]

        us = tiles("u", 3)
        a1s = tiles("a1_")
        a2s = tiles("a2_")
        a3s = tiles("a3_")
        c1s = tiles("c1_")
        c2s = tiles("c2_")
        f1s = tiles("f1_")
        t1s = tiles("t1_")
        t2s = tiles("t2_")
        # s12 results for all chunks live in ONE tile so sqrt can span
        # several chunks in a single activation
        s12all = E(nc.sbuf_tensor("s12all", [NPART, PPTS], f16))
        esall = E(nc.sbuf_tensor("esall", [NPART, PPTS], f16))

        sem_u = [E(nc.semaphore(f"sem_u{k}")) for k in range(NCHUNK)]
        sem_dve = E(nc.semaphore("sem_dve"))
        sem_act = E(nc.semaphore("sem_act"))
        sem_out = E(nc.semaphore("sem_out"))
        block = E(nc.Block())

        def cst(i):
            return ct[:, i:i + 1]

        def s12sl(k):
            return s12all[:, OFFS[k]:OFFS[k + 1]]

        # --- engine emit orders ---
        # DVE: groups in chunk order; s12_j two chunks behind (its wait on
        # ACT is then always satisfied). Everything except the last s12 is
        # emitted before the final G: while DVE waits for the last chunk's
        # DMA it drains the s12 backlog for free.
        dve_s12_after = {k: [] for k in range(NCHUNK)}
        for j in range(NCHUNK - 4):
            dve_s12_after[j + 2].append(j)
        dve_s12_after[NCHUNK - 2] += [NCHUNK - 4, NCHUNK - 3, NCHUNK - 2]
        dve_s12_after[NCHUNK - 1] += [NCHUNK - 1]
        # ACT: squares per chunk; sqrt group g after squares of chunk
        # grp_after[g] (chosen so its s12s are already emitted on DVE and
        # wide sqrts run while the last chunk's DMA is in flight)
        grp_after = {}
        for gi, (a, bb) in enumerate(SQRT_GROUPS):
            if bb >= NCHUNK:
                grp_after[gi] = NCHUNK      # trailing, after every square
            else:
                grp_after[gi] = min(bb + 1, NCHUNK - 2)
        # groups whose last s12 only appears after the final G: trailing
        act_order = []
        for k in range(NCHUNK):
            act_order.append(("sq", k))
            for gi, (a, bb) in enumerate(SQRT_GROUPS):
                if grp_after[gi] == k:
                    act_order.append(("grp", gi))
        emitted = {x[1] for x in act_order if x[0] == "grp"}
        for gi in range(len(SQRT_GROUPS)):
            if gi not in emitted:
                act_order.append(("grp", gi))

        # --- semaphore tick bookkeeping (program order per engine) ---
        dve_c2, dve_f1, dve_s12 = {}, {}, {}
        act_t1, act_grp = {}, {}
        dve_n = act_n = 0
        for k in range(NCHUNK):
            dve_n += 1; dve_c2[k] = dve_n
            dve_n += 1; dve_f1[k] = dve_n
            for j in dve_s12_after[k]:
                dve_n += 1; dve_s12[j] = dve_n
        for op, idx in act_order:
            act_n += 1
            if op == "sq":
                act_t1[idx] = act_n
            else:
                act_grp[idx] = act_n

        @block.gpsimd
        def _(g):
            g.dma_start(ct[:], consts).then_inc(sem_u[0], 16)
            for k in range(NCHUNK):
                u3d = us[k][:].rearrange("p (c f) -> p c f", c=3)
                poff = 0
                for Pf in PIECES[k]:
                    base = NPART * 3 * (OFFS[k] + poff)
                    g.dma_start(
                        u3d[:, :, poff:poff + Pf],
                        pc[base:base + 3 * NPART * Pf].rearrange(
                            "(c p f) -> p c f", c=3, p=NPART),
                    ).then_inc(sem_u[k], 16)
                    poff += Pf

        @block.vector
        def _(v):
            def combines(k):
                F = CHUNKS[k]
                u = us[k]
                u1, u2, u3 = (u[:, i * F:(i + 1) * F] for i in range(3))
                # chunk k's sem counts 16 per piece DMA
                # (+16 for the consts DMA on chunk 0)
                need = 16 * len(PIECES[k]) + (16 if k == 0 else 0)
                v.wait_ge(sem_u[k], need)
                # order keeps >=1 instruction between each same-engine RAW
                # pair: a2->c1 dist 2, a3->c2 dist 4, c1->f1 dist 2.
                v.tensor_scalar(a3s[k][:], u3, cst(2), None, Alu.mult)
                v.tensor_scalar(a2s[k][:], u3, cst(1), None, Alu.mult)
                v.tensor_scalar(a1s[k][:], u2, cst(0), None, Alu.mult)
                v.tensor_tensor(c1s[k][:], u1, a2s[k][:], Alu.add)
                v.tensor_tensor(c2s[k][:], u2, a3s[k][:],
                                Alu.add).then_inc(sem_dve, 1)
                v.tensor_tensor(f1s[k][:], c1s[k][:], a1s[k][:],
                                Alu.add).then_inc(sem_dve, 1)

            def s12(k):
                v.wait_ge(sem_act, act_t1[k])
                v.tensor_tensor(s12sl(k), t1s[k][:], t2s[k][:],
                                Alu.add).then_inc(sem_dve, 1)

            for k in range(NCHUNK):
                combines(k)
                for j in dve_s12_after[k]:
                    s12(j)

        @block.scalar
        def _(s):
            # dummy activations preload the Square/Sqrt tables while the
            # first DMA is still in flight
            s.activation(scr[:, 2:4], scr[:, 0:2], Act.Square)
            s.activation(scr[:, 0:2], scr[:, 0:2], Act.Sqrt)

            def squares(k):
                s.wait_ge(sem_dve, dve_c2[k])
                s.activation(t2s[k][:], c2s[k][:], Act.Square,
                             bias=cst(6), scale=cst(5))
                s.wait_ge(sem_dve, dve_f1[k])
                s.activation(t1s[k][:], f1s[k][:], Act.Square,
                             bias=cst(4), scale=cst(3)).then_inc(sem_act, 1)

            def sqrt_grp(gi):
                a, bb = SQRT_GROUPS[gi]
                dve_need = max(dve_s12[k] for k in range(a, bb))
                s.wait_ge(sem_dve, dve_need)
                s.activation(esall[:, OFFS[a]:OFFS[bb]],
                             s12all[:, OFFS[a]:OFFS[bb]], Act.Sqrt,
                             bias=cst(7),
                             accum_out=acc[:, gi:gi + 1]).then_inc(sem_act, 1)

            for op, idx in act_order:
                if op == "sq":
                    squares(idx)
                else:
                    sqrt_grp(idx)

        @block.sync
        def _(sp):
            # first half of the accumulator goes out as soon as its sqrt
            # groups are done; the rest right at the end
            half = ngroups - 1
            sp.wait_ge(sem_act, act_grp[half - 1])
            sp.dma_start(acc_out0, acc[:, 0:half]).then_inc(sem_out, 16)
            sp.wait_ge(sem_act, act_grp[ngroups - 1])
            sp.dma_start(acc_out1, acc[:, half:]).then_inc(sem_out, 16)
            sp.wait_ge(sem_out, 32)

    return nc


def _get_nc():
    global _CACHED_NC
    if _CACHED_NC is None:
        _CACHED_NC = _build_nc()
    return _CACHED_NC


def _kernel_impl(point_clouds, target_transl, target_rot, transl_err, rot_err,
                 trace=False):
    from concourse.bass_utils import run_bass_kernel_spmd

    pc = np.asarray(point_clouds)
    tt = np.asarray(target_transl, np.float64)
    tr = np.asarray(target_rot, np.float64)
    te = np.asarray(transl_err, np.float64)
    re_ = np.asarray(rot_err, np.float64)

    # ---- pose loss (host, float64, exact reference formulas) ----
    d = np.abs(te - tt)
    loss_transl = np.where(d < 1.0, 0.5 * d * d, d - 0.5).sum(axis=1).mean()

    rinv = tr * np.array([1.0, -1.0, -1.0, -1.0])
    q = re_
    w = q[:, 0]*rinv[:, 0] - q[:, 1]*rinv[:, 1] - q[:, 2]*rinv[:, 2] - q[:, 3]*rinv[:, 3]
    x = q[:, 0]*rinv[:, 1] + q[:, 1]*rinv[:, 0] + q[:, 2]*rinv[:, 3] - q[:, 3]*rinv[:, 2]
    y = q[:, 0]*rinv[:, 2] - q[:, 1]*rinv[:, 3] + q[:, 2]*rinv[:, 0] + q[:, 3]*rinv[:, 1]
    z = q[:, 0]*rinv[:, 3] + q[:, 1]*rinv[:, 2] - q[:, 2]*rinv[:, 1] + q[:, 3]*rinv[:, 0]
    angle = 2.0 * np.arctan2(np.sqrt(x*x + y*y + z*z), np.abs(w))
    loss_rot = (180.0 * angle / np.pi).mean()
    pose_loss = loss_transl + loss_rot

    # ---- per-sample transform constants (host) ----
    all_consts = np.zeros((B, NCONST), np.float32)
    all_piv = []
    for b in range(B):
        piv, consts = _per_sample_host(tt[b], tr[b], te[b], re_[b])
        all_consts[b] = consts
        all_piv.append(piv)

    # ---- build per-core inputs (permute coord rows per pivoting, pack
    #      4 samples x 32 partition-rows x 6250 points, chunk-major) ----
    pcp = np.stack([pc[b, all_piv[b], :] for b in range(B)])   # [B,3,N]
    pcp = pcp.reshape(NCORES, SPC, 3, ROWS, PPTS).transpose(0, 1, 3, 2, 4)
    pcp = pcp.reshape(NCORES, NPART, 3, PPTS)
    # piece-major flat: [concat over (chunk, piece) of [3, 128, Pf]]
    blocks = []
    for k in range(NCHUNK):
        poff = OFFS[k]
        for Pf in PIECES[k]:
            blocks.append(pcp[:, :, :, poff:poff + Pf]
                          .transpose(0, 2, 1, 3).reshape(NCORES, -1))
            poff += Pf
    pcf = np.concatenate(blocks, axis=1)
    in_maps = []
    for k in range(NCORES):
        cc = all_consts[k * SPC:(k + 1) * SPC]                 # [SPC, 8]
        in_maps.append({
            "pc": np.ascontiguousarray(pcf[k]),
            "consts": np.repeat(cc, ROWS, axis=0),             # [128, 8]
        })

    nc = _get_nc()
    res = run_bass_kernel_spmd(nc, in_maps, core_ids=list(range(NCORES)),
                               trace=trace)

    # ---- combine (host, float64) ----
    pcl_sum = 0.0
    for k in range(NCORES):
        pcl_sum += (res.results[k]["acc0"].astype(np.float64).sum()
                    + res.results[k]["acc1"].astype(np.float64).sum()) / N

    total = 0.5 * pose_loss + 0.5 * (pcl_sum / B)
    out = (np.float32(total), np.float32(loss_transl), np.float32(loss_rot),
           np.float32(pcl_sum / B))
    return out, res


def kernel(point_clouds, target_transl, target_rot, transl_err, rot_err):
    out, _ = _kernel_impl(point_clouds, target_transl, target_rot,
                          transl_err, rot_err)
    return out
